# revision 32
# baseline (speedup 1.0000x reference)
"""Block-sparse linear kernel for Trainium2 (8 NeuronCores, SPMD data-parallel).

Computes y = x @ (W * mask) + bias for
    x    [8, 1024, 4096] f32
    W    [4096, 4096]    f32
    mask [4096, 4096]    int32 (32x32-block structured, ~25% block density)
    bias [4096]          f32
    y    [8, 1024, 4096] f32

Strategy
--------
- Data parallel: core c computes rows [1024c, 1024(c+1)) of the flattened
  [8192, 4096] activation (i.e. batch element c).
- The trn2 PE array runs in 64x32 tiling mode (8 concurrent sub-arrays).
  The mask's 32x32 block granularity maps onto vertical block pairs: each
  present 64x32 "super cell" (block rows 2I,2I+1 x block col j, present if
  either 32x32 block is nonzero) becomes one K=64/M=32/N=512 matmul on
  sub-array (row_grp=I%2, col_grp=j%4).  Rows are permuted (max-weight
  matching) so paired rows co-occur in many columns, minimizing supercells.
- Stationary-weight reuse: the per-core output has 1024 rows = 2 PSUM
  m-slices of 512.  For the bulk of the supertiles, each weight panel is
  loaded into the PE once (LDWEIGHTS) and used by TWO matmuls (m=0 and
  m=1).  The Bass tile scheduler emits one InstLdweights per matmul
  unconditionally, so after scheduling we delete the redundant second
  LDWEIGHTS (the hardware keeps per-position stationary weights; verified
  on hardware).  The m0/m1 matmuls of one panel are separated by the other
  7 sub-array positions' matmuls so the in-order PE queue never stalls on
  a busy sub-array.
- Ramp: x arrives over ~50us of DMA; the first N_PRE supertiles run
  m=0-only while x m1 streams in (the first 4 in a chunk-major merged
  order that tracks x-chunk arrival).  Their weight tiles stay resident in
  SBUF and their m=1 matmuls run at the end (phase 3) with fresh
  LDWEIGHTS.
- Weights stream from DRAM exactly once (plus nothing for the ramp tiles,
  which stay resident); x is transposed/cast host-side; all matmul FLOPs
  run in bf16 with fp32 PSUM accumulation (measured rel. error ~2e-3).
"""

import numpy as np
import ml_dtypes

B, S, IN_F, OUT_F = 8, 1024, 4096, 4096
BS = 32                      # sparsity block size
GI, GJ = IN_F // BS, OUT_F // BS
GP = GI // 2                 # vertical super-rows (64 rows each)
N_CORES = 8
M_CORE = (B * S) // N_CORES  # rows of x per core (1024)
MSL = 512                    # m-slice width (one PSUM bank of fp32)
N_MSL = M_CORE // MSL        # 2
JCOLS = 4                    # output block-columns per supertile (4*32 = 128 partitions)
N_J = GJ // JCOLS            # 32 output supertiles
N_T = IN_F // 128            # 32 xT tiles
N_PRE = 10                   # ramp supertiles (m0 first, m1 in phase 3)

BF16 = ml_dtypes.bfloat16


def _ensure_ntff_hook():
    """Best-effort: make trace=True work under axon when the image's antenv
    lacks axon_hooks.  Harmless if it fails — tracing is skipped, results
    are still correct."""
    import sys, types
    try:
        import antenv  # noqa
    except ImportError:
        return
    try:
        from antenv.axon_hooks import get_axon_ntff_profile_hook
        if get_axon_ntff_profile_hook() is not None:
            return
        mod = sys.modules["antenv.axon_hooks"]
    except ImportError:
        mod = types.ModuleType("antenv.axon_hooks")
        mod._hook = None
        def set_axon_ntff_profile_hook(h, _m=mod):
            _m._hook = h
        def get_axon_ntff_profile_hook(_m=mod):
            return _m._hook
        mod.set_axon_ntff_profile_hook = set_axon_ntff_profile_hook
        mod.get_axon_ntff_profile_hook = get_axon_ntff_profile_hook
        sys.modules["antenv.axon_hooks"] = mod
        import antenv as _a
        _a.axon_hooks = mod
    try:
        from trn_agent_boot.trn_boot import _ntff_profile_via_ctypes
        mod.set_axon_ntff_profile_hook(
            _ntff_profile_via_ctypes("/opt/axon/libaxon_pjrt.so")
        )
    except Exception:
        pass


def _pair_permutation(nzb):
    """Order block-rows so vertically-paired rows co-occur in many columns.

    Greedy max-weight matching on C[a,b] = #columns where blocks a and b are
    both present; each matched pair becomes one 64-row super-row, so high
    weight = fewer half-empty 64x32 panels = fewer matmuls.
    """
    C = nzb.astype(np.int32) @ nzb.astype(np.int32).T
    pairs = []
    try:
        import networkx as nx
        G = nx.Graph()
        for a in range(GI):
            for b in range(a + 1, GI):
                G.add_edge(a, b, weight=int(C[a, b]))
        pairs = [
            (int(min(a, b)), int(max(a, b)))
            for a, b in nx.max_weight_matching(G, maxcardinality=True)
        ]
    except Exception:
        pass
    if len(pairs) != GI // 2:
        pairs = []
        iu = np.triu_indices(GI, k=1)
        order = np.argsort(C[iu])[::-1]
        used = np.zeros(GI, dtype=bool)
        for idx in order:
            a, b = iu[0][idx], iu[1][idx]
            if not used[a] and not used[b]:
                used[a] = used[b] = True
                pairs.append((int(a), int(b)))
                if len(pairs) == GI // 2:
                    break
    perm = []
    for a, b in pairs:
        perm.extend((a, b))
    for a in range(GI):      # safety for odd leftovers
        if a not in perm:
            perm.append(a)
    return np.asarray(perm)


def _plan(nzb):
    """Per-supertile weight storage layout and MM schedule (64x32 pairing).

    nzb: bool [GI, GJ] — which 32x32 blocks are present (in permuted row
    order).

    Returns (plan, strip_cols):
      plan[J] = {
        'chunks': {r2: (src_col_base, n_cells)},            # DMA per row strip
        'sched':  [(r2, c, woff_or_None, I, start, stop)],  # round-robin
        'rounds': [[entry, ...], ...],                      # same, grouped
      }
      strip_cols[r2] = total columns of strip r2's DRAM panel (r2 in {0,1}).
    woff None => dummy matmul with the zero-weight tile (region had no cells
    but must be initialized so the bank reduce reads defined values).
    """
    nzb_top, nzb_bot = nzb[0::2], nzb[1::2]
    nzb2 = nzb_top | nzb_bot           # [GP, GJ] supercell presence
    plan = []
    strip_cols = [0, 0]                # 64-row tensor (legacy + pairs) cols
    single_cols = [0, 0]               # 32-row singles tensor cols
    for J in range(N_J):
        legacy = J < 4                 # GEN0 keeps x-ascending padded layout
        chunks = {}
        queues = {}                    # (r2, c) -> list of (r2, c, woff, I)
        for r2 in range(2):
            # x-tile-ascending iteration (the GEN0 ramp consumes x chunks as
            # they arrive)
            cl = [(I, j)
                  for I in range(GP) if I % 2 == r2
                  for j in range(J * JCOLS, (J + 1) * JCOLS) if nzb2[I, j]]
            if legacy:
                order = cl
                chunks[r2] = ("legacy", strip_cols[r2], len(cl))
                strip_cols[r2] += len(cl) * BS
            else:
                # pairs keep 64-row panels; half-present cells store only the
                # present 32-row half (the absent half is memset to zero in
                # SBUF), cutting the zero-padding out of the DMA stream
                pairs = [e for e in cl if nzb_top[e[0], e[1]] and nzb_bot[e[0], e[1]]]
                tops = [e for e in cl if nzb_top[e[0], e[1]] and not nzb_bot[e[0], e[1]]]
                bots = [e for e in cl if not nzb_top[e[0], e[1]]]
                order = pairs + tops + bots
                chunks[r2] = ("compact", strip_cols[r2], len(pairs),
                              single_cols[r2], len(tops), len(bots))
                strip_cols[r2] += len(pairs) * BS
                single_cols[r2] += (len(tops) + len(bots)) * BS
            for k, (I, j) in enumerate(order):
                c = j % 4
                queues.setdefault((r2, c), []).append((r2, c, k * BS, I))
        for r2 in range(2):
            for c in range(4):
                if (r2, c) not in queues:
                    queues[(r2, c)] = [(r2, c, None, 0)]
        # Round-robin across the 8 sub-array positions for concurrency.
        sched = []
        rounds = []
        qlists = [queues[k] for k in sorted(queues.keys())]
        idx = [0] * len(qlists)
        remaining = sum(len(q) for q in qlists)
        while remaining:
            rnd = []
            for qi, q in enumerate(qlists):
                if idx[qi] < len(q):
                    r2, c, woff, I = q[idx[qi]]
                    start = idx[qi] == 0
                    stop = idx[qi] == len(q) - 1
                    e = (r2, c, woff, I, start, stop)
                    sched.append(e)
                    rnd.append(e)
                    idx[qi] += 1
                    remaining -= 1
            rounds.append(rnd)
        plan.append({"chunks": chunks, "sched": sched, "rounds": rounds})
    return plan, strip_cols, single_cols


def _strip_reused_ldweights(nc, pairs):
    """Delete the scheduler-generated InstLdweights immediately preceding
    each 'reuse' matmul.

    pairs: list of (keeper_mm_name, reuse_mm_name).  The reuse matmul sits
    at the same PE tile position as the keeper with an identical stationary
    AP and no intervening LDWEIGHTS at that position, so the hardware's
    per-position stationary weights are still valid (verified on hw).
    Any waits/updates on the deleted LDW are transferred to the matmul.
    """
    import concourse.mybir as mybir

    reuse_of = {r: k for (k, r) in pairs}
    removed = set()
    for fn in nc.m.functions:
        for blk in fn.blocks:
            ilist = list(blk.instructions)
            name_to_idx = {inst.name: i for i, inst in enumerate(ilist)}

            def ldw_before(i):
                j = i - 1
                while j >= 0 and not isinstance(
                    ilist[j], (mybir.InstLdweights, mybir.InstMatmult)
                ):
                    j -= 1
                assert j >= 0 and isinstance(ilist[j], mybir.InstLdweights), (
                    f"no LDWEIGHTS before {ilist[i].name}"
                )
                return j

            to_del = []
            for reuse_nm, keep_nm in reuse_of.items():
                i = name_to_idx.get(reuse_nm)
                if i is None:
                    continue
                mm = ilist[i]
                j = ldw_before(i)
                ldw = ilist[j]
                k = name_to_idx.get(keep_nm)
                assert k is not None and k < j, (reuse_nm, keep_nm)
                jk = ldw_before(k)
                ldw_keep = ilist[jk]
                # safety: identical position and stationary source
                assert ldw.tile_position == ldw_keep.tile_position, (
                    ldw.tile_position, ldw_keep.tile_position)
                assert ldw.tile_size == ldw_keep.tile_size
                a, b = ldw.ins[0], ldw_keep.ins[0]
                assert a.offset == b.offset and str(a) == str(b), (
                    reuse_nm, str(a), str(b))
                si = ldw.sync_info
                if si is not None and (len(si.on_wait) or len(si.on_update)):
                    msi = mm.sync_info
                    ow = list(si.on_wait) + (list(msi.on_wait) if msi else [])
                    ou = list(si.on_update) + (list(msi.on_update) if msi else [])
                    mm.sync_info = mybir.SyncInfo(on_wait=ow, on_update=ou)
                to_del.append(j)
                removed.add(ldw.name)
            for j in sorted(to_del, reverse=True):
                del blk.instructions[j]
            if removed:
                for inst in blk.instructions:
                    try:
                        edges = inst.dependency_edges()
                    except Exception:
                        continue
                    for edge in edges:
                        n = edge[0]
                        if n in removed:
                            inst.try_remove_dependency(n)
    return len(removed)


def _chunk_width(ch):
    """SBUF columns a (J, r2) strip occupies."""
    if ch[0] == "legacy":
        return ch[2] * BS
    return (ch[2] + ch[4] + ch[5]) * BS


def _build_program(plan, strip_cols, single_cols):
    import concourse.bacc as bacc
    import concourse.tile as tile
    import concourse.mybir as mybir

    nc = bacc.Bacc(debug=False)
    bf16, f32 = mybir.dt.bfloat16, mybir.dt.float32

    xt_d = nc.declare_dram_parameter(
        "xt", [N_MSL * (N_T // 2), 128, 2 * MSL], bf16, isOutput=False
    )
    w_d = {}
    ws_d = {}
    for r2 in range(2):
        if strip_cols[r2] > 0:
            w_d[r2] = nc.declare_dram_parameter(
                f"w{r2}", [2 * BS, strip_cols[r2]], bf16, isOutput=False
            )
        if single_cols[r2] > 0:
            ws_d[r2] = nc.declare_dram_parameter(
                f"ws{r2}", [BS, single_cols[r2]], bf16, isOutput=False
            )
    out_d = nc.declare_dram_parameter("out", [OUT_F, M_CORE], bf16, isOutput=True)

    # Largest per-(J, strip) weight chunk, in columns (>= BS for the tile alloc).
    lmax = BS
    for p in plan:
        for r2 in range(2):
            lmax = max(lmax, _chunk_width(p["chunks"][r2]))

    reuse_pairs = []  # (keeper_mm_name, reuse_mm_name) for LDW dedup

    with tile.TileContext(nc) as tc:
        with (
            tc.tile_pool(name="xp", bufs=1) as xp,
            tc.tile_pool(name="zp", bufs=1) as zp,
            tc.tile_pool(name="wr", bufs=1) as wr,     # resident ramp weights
            tc.tile_pool(name="wp", bufs=6) as wp,     # streaming phase-2 weights
            tc.tile_pool(name="ep", bufs=16) as ep,
            tc.tile_pool(name="pp", bufs=4, space="PSUM") as pp,
        ):
            wts = {}

            def load_w(J, eng0=None, eng1=None):
                if J < N_PRE:
                    width = max(BS, max(_chunk_width(plan[J]["chunks"][r2])
                                        for r2 in range(2)))
                    wt = wr.tile([128, width], bf16, tag=f"w{J}")
                else:
                    wt = wp.tile([128, lmax], bf16, tag="wt")
                engs = (eng0 or nc.sync, eng1 or eng0 or nc.scalar)
                # zero the absent halves of half-present cells before the DMAs
                for r2 in range(2):
                    ch = plan[J]["chunks"][r2]
                    if ch[0] == "compact" and ch[4] + ch[5]:
                        nc.vector.memset(
                            wt[64 * r2 : 64 * r2 + 64,
                               ch[2] * BS : (ch[2] + ch[4] + ch[5]) * BS],
                            0.0,
                        )
                for r2 in range(2):
                    ch = plan[J]["chunks"][r2]
                    if ch[0] == "legacy":
                        _, base, ncell = ch
                        if ncell:
                            engs[r2].dma_start(
                                wt[64 * r2 : 64 * r2 + 64, : ncell * BS],
                                w_d[r2][:, base : base + ncell * BS],
                            )
                        continue
                    _, pbase, npair, sbase, ntop, nbot = ch
                    if npair:
                        engs[r2].dma_start(
                            wt[64 * r2 : 64 * r2 + 64, : npair * BS],
                            w_d[r2][:, pbase : pbase + npair * BS],
                        )
                    if ntop:
                        engs[r2].dma_start(
                            wt[64 * r2 : 64 * r2 + 32,
                               npair * BS : (npair + ntop) * BS],
                            ws_d[r2][:, sbase : sbase + ntop * BS],
                        )
                    if nbot:
                        engs[r2].dma_start(
                            wt[64 * r2 + 32 : 64 * r2 + 64,
                               (npair + ntop) * BS : (npair + ntop + nbot) * BS],
                            ws_d[r2][:, sbase + ntop * BS
                                     : sbase + (ntop + nbot) * BS],
                        )
                wts[J] = wt
                return wt

            Xc = {}

            def load_x_chunk(t, m, eng):
                # one DMA per chunk PAIR (2t, 2t+1): per-partition source runs
                # are 2 KB contiguous (host stores pairs adjacently), halving
                # descriptor count vs per-chunk loads
                tp = t // 2
                if (tp, m) in Xc:
                    return
                xchunk = xp.tile([128, 2 * MSL], bf16, tag=f"x{tp}_{m}")
                Xc[(tp, m)] = xchunk
                eng.dma_start(xchunk[:], xt_d[m * (N_T // 2) + tp])

            # DMA emission order.  Per-queue FIFO plan (queues measured at
            # ~150-240 GB/s each; HBM ~330 GB/s aggregate):
            #   sync:   GEN0 heads | GEN0 tails(J0,1) | w4-7  | xm1 tail | w12,14,..
            #   scalar: xm0 evens  | GEN0 tail(J2)    | w8-11 | xm1 share | w13,15,..
            #   gpsimd: xm0 odds   | GEN0 tail(J3)    | xm1 bulk | outputs
            # GEN0 strips are split into a head (the cells the chunk-major
            # ramp consumes first) + remainder so the first matmuls' weights
            # land ASAP.
            heads = {}

            def load_w_head(J, ncut):
                width = max(BS, max(_chunk_width(plan[J]["chunks"][r2])
                                    for r2 in range(2)))
                wt = wr.tile([128, width], bf16, tag=f"w{J}")
                for r2 in range(2):
                    _, base, ncell = plan[J]["chunks"][r2]
                    n0 = min(ncut, ncell)
                    heads[(J, r2)] = n0
                    if n0:
                        nc.sync.dma_start(
                            wt[64 * r2 : 64 * r2 + 64, : n0 * BS],
                            w_d[r2][:, base : base + n0 * BS],
                        )
                wts[J] = wt
                return wt

            def load_w_tail(J, eng):
                wt = wts[J]
                for r2 in range(2):
                    _, base, ncell = plan[J]["chunks"][r2]
                    n0 = heads[(J, r2)]
                    if ncell > n0:
                        eng.dma_start(
                            wt[64 * r2 : 64 * r2 + 64, n0 * BS : ncell * BS],
                            w_d[r2][:, base + n0 * BS : base + ncell * BS],
                        )

            for J in range(4):
                load_w_head(J, 20)
            for tp in range(0, 4):
                load_x_chunk(2 * tp, 0, (nc.scalar, nc.gpsimd)[tp % 2])
            load_w_tail(0, nc.sync)
            load_w_tail(1, nc.sync)
            load_w_tail(2, nc.sync)
            load_w_tail(3, nc.sync)
            for tp in range(4, N_T // 2):
                load_x_chunk(2 * tp, 0, (nc.scalar, nc.gpsimd)[tp % 2])
            zw = zp.tile([128, BS], bf16)
            nc.vector.memset(zw[:], 0.0)
            # ramp weights: sync only (x owns scalar+gpsimd)
            for J in range(4, N_PRE):
                load_w(J, nc.sync, nc.sync)
            # x m1 right behind x m0 on both x queues
            for tp in range(0, N_T // 2):
                load_x_chunk(2 * tp, 1, (nc.scalar, nc.gpsimd)[tp % 2])
            # first two phase-2 weight tiles early, so the fused phase doesn't
            # wait on them right after the ramp
            load_w(N_PRE, nc.sync, nc.sync)
            load_w(N_PRE + 1, nc.sync, nc.sync)
            # remaining phase-2 weights are emitted just-in-time inside the
            # loop below (wp pool rotation paces them ~6 supertiles ahead).

            def emit_mm(P, wt, r2, c, woff, I, m, start, stop):
                lhsT = (
                    zw[64 * r2 : 64 * r2 + 64, :]
                    if woff is None
                    else wt[64 * r2 : 64 * r2 + 64, woff : woff + BS]
                )
                tlo = (I // 2) % 2
                return nc.tensor.matmul(
                    P[32 * c : 32 * c + 32, r2, :],
                    lhsT,
                    Xc[(I // 4, m)][64 * r2 : 64 * r2 + 64,
                                    tlo * MSL : (tlo + 1) * MSL],
                    start=start,
                    stop=stop,
                    tile_position=(64 * r2, 32 * c),
                )

            def emit_evac(P, J, m, eng=None):
                # ob is bf16: halves output DMA traffic; the final-value
                # rounding adds ~1e-3 relative error (budget is 2e-2).
                ob = ep.tile([128, MSL], bf16, tag="ob")
                with nc.allow_low_precision(
                    reason="final-value bf16 rounding; rel err budget 2e-2"
                ):
                    nc.vector.reduce_sum(
                        ob[:], P[:].transpose([0, 2, 1]), axis=mybir.AxisListType.X
                    )
                if eng is None:
                    eng = nc.gpsimd
                eng.dma_start(
                    out_d[128 * J : 128 * (J + 1), m * MSL : (m + 1) * MSL],
                    ob[:],
                )

            # --- phase 1a: ramp generation, J=0..3 m0, chunk-major merged ---
            # All four tiles' blocks for x chunk t run before any of chunk
            # t+1, using all 4 PSUM slots, so compute tracks x-chunk arrival
            # instead of head-of-line blocking on one tile's late chunks.
            GEN0 = list(range(min(4, N_PRE)))
            merged = []
            for J in GEN0:
                for k, (r2, c, woff, I, _s0, _s1) in enumerate(plan[J]["sched"]):
                    t = -1 if woff is None else I // 2
                    merged.append((t, k, J, r2, c, woff, I))
            merged.sort(key=lambda e: (e[0], e[1], e[2]))
            first_of = {}
            last_of = {}
            for idx, e in enumerate(merged):
                key = (e[2], e[3], e[4])
                first_of.setdefault(key, idx)
                last_of[key] = idx

            P_gen = {J: pp.tile([128, 2, MSL], f32, tag="P", name=f"Pg{J}")
                     for J in GEN0}
            for idx, (t, k, J, r2, c, woff, I) in enumerate(merged):
                key = (J, r2, c)
                emit_mm(
                    P_gen[J], wts[J], r2, c, woff, I, 0,
                    first_of[key] == idx, last_of[key] == idx,
                )
            for J in GEN0:
                emit_evac(P_gen[J], J, 0)

            # --- phase 1b: ramp J=4..N_PRE-1, m0 only ---
            for J in range(len(GEN0), N_PRE):
                P = pp.tile([128, 2, MSL], f32, tag="P")
                for r2, c, woff, I, start, stop in plan[J]["sched"]:
                    emit_mm(P, wts[J], r2, c, woff, I, 0, start, stop)
                emit_evac(P, J, 0)

            # --- phase 2: J=N_PRE..N_J-1, fused m0+m1 with stationary reuse.
            # Per round-robin round: all positions' m0 matmuls, then the same
            # entries' m1 matmuls (same stationary weights; the second LDW is
            # deleted after scheduling).  Same-position instructions stay ~8
            # apart so the in-order PE queue never waits on a busy sub-array.
            for J in range(N_PRE, N_J):
                if J in wts:
                    wt = wts[J]
                else:
                    eng = (nc.sync, nc.scalar)[J % 2]
                    wt = load_w(J, eng, eng)
                P0 = pp.tile([128, 2, MSL], f32, tag="P")
                P1 = pp.tile([128, 2, MSL], f32, tag="P")
                rounds = plan[J]["rounds"]
                for ri, rnd in enumerate(rounds):
                    mm0 = [emit_mm(P0, wt, r2, c, woff, I, 0, start, stop)
                           for (r2, c, woff, I, start, stop) in rnd]
                    if ri == len(rounds) - 1:
                        emit_evac(P0, J, 0)
                    mm1 = [emit_mm(P1, wt, r2, c, woff, I, 1, start, stop)
                           for (r2, c, woff, I, start, stop) in rnd]
                    for a, b in zip(mm0, mm1):
                        reuse_pairs.append((a.ins.name, b.ins.name))
                emit_evac(P1, J, 1)

            # --- phase 3: ramp tiles' m1, weights still resident in SBUF ---
            for J in range(N_PRE):
                P = pp.tile([128, 2, MSL], f32, tag="P")
                for r2, c, woff, I, start, stop in plan[J]["sched"]:
                    emit_mm(P, wts[J], r2, c, woff, I, 1, start, stop)
                emit_evac(P, J, 1)

    n = _strip_reused_ldweights(nc, reuse_pairs)
    assert n == len(reuse_pairs), (n, len(reuse_pairs))
    nc.compile()
    return nc


_CACHE = {}


def kernel(x, W, bias, mask):
    assert x.shape == (B, S, IN_F) and W.shape == (IN_F, OUT_F)
    _ensure_ntff_hook()
    from concourse.bass_utils import run_bass_kernel_spmd

    # --- host-side input prep -------------------------------------------
    mask_nz = mask != 0
    nzb = np.asarray(mask_nz.reshape(GI, BS, GJ, BS).any(axis=(1, 3)))

    key = nzb.tobytes()
    if key not in _CACHE:
        perm = _pair_permutation(nzb)
        plan, strip_cols, single_cols = _plan(nzb[perm])
        nc = _build_program(plan, strip_cols, single_cols)
        _CACHE[key] = (perm, plan, strip_cols, single_cols, nc)
    perm, plan, strip_cols, single_cols, nc = _CACHE[key]
    nzb_p = nzb[perm]

    # Masked weights, gathered per row strip in storage order (J-major),
    # mirroring _plan's per-(J, r2) cell order exactly.
    Wm = np.where(mask_nz, W, np.float32(0)).astype(np.float32)
    W4 = Wm.reshape(GI, BS, GJ, BS)  # block (i, j) = W4[i, :, j, :]
    nzb_top, nzb_bot = nzb_p[0::2], nzb_p[1::2]
    nzb2 = nzb_top | nzb_bot
    strips = {}
    sstrips = {}
    for r2 in range(2):
        pair_II, pair_JJ = [], []      # 64-row panels (legacy + pairs)
        sing_II, sing_JJ = [], []      # 32-row panels (block-row index)
        for J in range(N_J):
            cl = [(I, j)
                  for I in range(GP) if I % 2 == r2
                  for j in range(J * JCOLS, (J + 1) * JCOLS) if nzb2[I, j]]
            if J < 4:
                for I, j in cl:
                    pair_II.append(I)
                    pair_JJ.append(j)
            else:
                for I, j in cl:
                    if nzb_top[I, j] and nzb_bot[I, j]:
                        pair_II.append(I)
                        pair_JJ.append(j)
                for I, j in cl:
                    if nzb_top[I, j] and not nzb_bot[I, j]:
                        sing_II.append(perm[2 * I])
                        sing_JJ.append(j)
                for I, j in cl:
                    if not nzb_top[I, j]:
                        sing_II.append(perm[2 * I + 1])
                        sing_JJ.append(j)
        if pair_II:
            II = np.asarray(pair_II)
            JJ = np.asarray(pair_JJ)
            top = W4[perm[2 * II], :, JJ, :]       # [n, 32, 32]
            bot = W4[perm[2 * II + 1], :, JJ, :]   # [n, 32, 32]
            panel = np.concatenate([top, bot], axis=1)  # [n, 64, 32]
            strips[r2] = np.ascontiguousarray(
                panel.transpose(1, 0, 2).reshape(2 * BS, -1)
            ).astype(BF16)
        if sing_II:
            pan = W4[np.asarray(sing_II), :, np.asarray(sing_JJ), :]  # [n,32,32]
            sstrips[r2] = np.ascontiguousarray(
                pan.transpose(1, 0, 2).reshape(BS, -1)
            ).astype(BF16)
        assert strips.get(r2, np.zeros((1, 0))).shape[1] == strip_cols[r2]
        assert sstrips.get(r2, np.zeros((1, 0))).shape[1] == single_cols[r2]

    xf = np.ascontiguousarray(x).reshape(B * S, IN_F)
    in_maps = []
    for c in range(N_CORES):
        xt = np.ascontiguousarray(
            xf[c * M_CORE : (c + 1) * M_CORE].T
        ).astype(BF16)
        xt = xt.reshape(GI, BS, M_CORE)[perm].reshape(IN_F, M_CORE)
        # [m][t//2][p][(t%2)*MSL + q]: chunk pairs adjacent per partition so
        # one DMA covers two chunks with 2KB-contiguous per-partition runs
        xtc = (
            xt.reshape(N_T // 2, 2, 128, N_MSL, MSL)
            .transpose(3, 0, 2, 1, 4)
            .reshape(N_MSL * (N_T // 2), 128, 2 * MSL)
        )
        m = {"xt": np.ascontiguousarray(xtc)}
        for r2, arr in strips.items():
            m[f"w{r2}"] = arr
        for r2, arr in sstrips.items():
            m[f"ws{r2}"] = arr
        in_maps.append(m)

    # --- run -------------------------------------------------------------
    res = run_bass_kernel_spmd(nc, in_maps, list(range(N_CORES)), trace=True)

    # --- host-side output assembly --------------------------------------
    y = np.empty((B * S, OUT_F), dtype=np.float32)
    for c in range(N_CORES):
        y[c * M_CORE : (c + 1) * M_CORE] = res.results[c]["out"].astype(np.float32).T
    y = y.reshape(B, S, OUT_F)
    if np.any(bias):
        # bias is all-zero in this problem's setup; handled host-side for
        # generality.
        y = y + bias.astype(np.float32)
    kernel.last_exec_time_ns = res.exec_time_ns
    return y


# revision 33
# speedup vs baseline: 1.0155x; 1.0155x over previous
"""Block-sparse linear kernel for Trainium2 (8 NeuronCores, SPMD data-parallel).

Computes y = x @ (W * mask) + bias for
    x    [8, 1024, 4096] f32
    W    [4096, 4096]    f32
    mask [4096, 4096]    int32 (32x32-block structured, ~25% block density)
    bias [4096]          f32
    y    [8, 1024, 4096] f32

Strategy
--------
- Data parallel: core c computes rows [1024c, 1024(c+1)) of the flattened
  [8192, 4096] activation (i.e. batch element c).
- The trn2 PE array runs in 64x32 tiling mode (8 concurrent sub-arrays).
  The mask's 32x32 block granularity maps onto vertical block pairs: each
  present 64x32 "super cell" (block rows 2I,2I+1 x block col j, present if
  either 32x32 block is nonzero) becomes one K=64/M=32/N=512 matmul on
  sub-array (row_grp=I%2, col_grp=j%4).  Rows are permuted (max-weight
  matching) so paired rows co-occur in many columns, minimizing supercells.
- Stationary-weight reuse: the per-core output has 1024 rows = 2 PSUM
  m-slices of 512.  For the bulk of the supertiles, each weight panel is
  loaded into the PE once (LDWEIGHTS) and used by TWO matmuls (m=0 and
  m=1).  The Bass tile scheduler emits one InstLdweights per matmul
  unconditionally, so after scheduling we delete the redundant second
  LDWEIGHTS (the hardware keeps per-position stationary weights; verified
  on hardware).  The m0/m1 matmuls of one panel are separated by the other
  7 sub-array positions' matmuls so the in-order PE queue never stalls on
  a busy sub-array.
- Ramp: x arrives over ~50us of DMA; the first N_PRE supertiles run
  m=0-only while x m1 streams in (the first 4 in a chunk-major merged
  order that tracks x-chunk arrival).  Their weight tiles stay resident in
  SBUF and their m=1 matmuls run at the end (phase 3) with fresh
  LDWEIGHTS.
- Weights stream from DRAM exactly once (plus nothing for the ramp tiles,
  which stay resident); x is transposed/cast host-side; all matmul FLOPs
  run in bf16 with fp32 PSUM accumulation (measured rel. error ~2e-3).
"""

import numpy as np
import ml_dtypes

B, S, IN_F, OUT_F = 8, 1024, 4096, 4096
BS = 32                      # sparsity block size
GI, GJ = IN_F // BS, OUT_F // BS
GP = GI // 2                 # vertical super-rows (64 rows each)
N_CORES = 8
M_CORE = (B * S) // N_CORES  # rows of x per core (1024)
MSL = 512                    # m-slice width (one PSUM bank of fp32)
N_MSL = M_CORE // MSL        # 2
JCOLS = 4                    # output block-columns per supertile (4*32 = 128 partitions)
N_J = GJ // JCOLS            # 32 output supertiles
N_T = IN_F // 128            # 32 xT tiles
N_PRE = 12                   # ramp supertiles (m0 first, m1 in phase 3)

BF16 = ml_dtypes.bfloat16


def _ensure_ntff_hook():
    """Best-effort: make trace=True work under axon when the image's antenv
    lacks axon_hooks.  Harmless if it fails — tracing is skipped, results
    are still correct."""
    import sys, types
    try:
        import antenv  # noqa
    except ImportError:
        return
    try:
        from antenv.axon_hooks import get_axon_ntff_profile_hook
        if get_axon_ntff_profile_hook() is not None:
            return
        mod = sys.modules["antenv.axon_hooks"]
    except ImportError:
        mod = types.ModuleType("antenv.axon_hooks")
        mod._hook = None
        def set_axon_ntff_profile_hook(h, _m=mod):
            _m._hook = h
        def get_axon_ntff_profile_hook(_m=mod):
            return _m._hook
        mod.set_axon_ntff_profile_hook = set_axon_ntff_profile_hook
        mod.get_axon_ntff_profile_hook = get_axon_ntff_profile_hook
        sys.modules["antenv.axon_hooks"] = mod
        import antenv as _a
        _a.axon_hooks = mod
    try:
        from trn_agent_boot.trn_boot import _ntff_profile_via_ctypes
        mod.set_axon_ntff_profile_hook(
            _ntff_profile_via_ctypes("/opt/axon/libaxon_pjrt.so")
        )
    except Exception:
        pass


def _pair_permutation(nzb):
    """Order block-rows so vertically-paired rows co-occur in many columns.

    Greedy max-weight matching on C[a,b] = #columns where blocks a and b are
    both present; each matched pair becomes one 64-row super-row, so high
    weight = fewer half-empty 64x32 panels = fewer matmuls.
    """
    C = nzb.astype(np.int32) @ nzb.astype(np.int32).T
    pairs = []
    try:
        import networkx as nx
        G = nx.Graph()
        for a in range(GI):
            for b in range(a + 1, GI):
                G.add_edge(a, b, weight=int(C[a, b]))
        pairs = [
            (int(min(a, b)), int(max(a, b)))
            for a, b in nx.max_weight_matching(G, maxcardinality=True)
        ]
    except Exception:
        pass
    if len(pairs) != GI // 2:
        pairs = []
        iu = np.triu_indices(GI, k=1)
        order = np.argsort(C[iu])[::-1]
        used = np.zeros(GI, dtype=bool)
        for idx in order:
            a, b = iu[0][idx], iu[1][idx]
            if not used[a] and not used[b]:
                used[a] = used[b] = True
                pairs.append((int(a), int(b)))
                if len(pairs) == GI // 2:
                    break
    perm = []
    for a, b in pairs:
        perm.extend((a, b))
    for a in range(GI):      # safety for odd leftovers
        if a not in perm:
            perm.append(a)
    return np.asarray(perm)


def _plan(nzb):
    """Per-supertile weight storage layout and MM schedule (64x32 pairing).

    nzb: bool [GI, GJ] — which 32x32 blocks are present (in permuted row
    order).

    Returns (plan, strip_cols):
      plan[J] = {
        'chunks': {r2: (src_col_base, n_cells)},            # DMA per row strip
        'sched':  [(r2, c, woff_or_None, I, start, stop)],  # round-robin
        'rounds': [[entry, ...], ...],                      # same, grouped
      }
      strip_cols[r2] = total columns of strip r2's DRAM panel (r2 in {0,1}).
    woff None => dummy matmul with the zero-weight tile (region had no cells
    but must be initialized so the bank reduce reads defined values).
    """
    nzb_top, nzb_bot = nzb[0::2], nzb[1::2]
    nzb2 = nzb_top | nzb_bot           # [GP, GJ] supercell presence
    plan = []
    strip_cols = [0, 0]                # 64-row tensor (legacy + pairs) cols
    single_cols = [0, 0]               # 32-row singles tensor cols
    for J in range(N_J):
        legacy = J < 4                 # GEN0 keeps x-ascending padded layout
        chunks = {}
        queues = {}                    # (r2, c) -> list of (r2, c, woff, I)
        for r2 in range(2):
            # x-tile-ascending iteration (the GEN0 ramp consumes x chunks as
            # they arrive)
            cl = [(I, j)
                  for I in range(GP) if I % 2 == r2
                  for j in range(J * JCOLS, (J + 1) * JCOLS) if nzb2[I, j]]
            if legacy:
                order = cl
                chunks[r2] = ("legacy", strip_cols[r2], len(cl))
                strip_cols[r2] += len(cl) * BS
            else:
                # pairs keep 64-row panels; half-present cells store only the
                # present 32-row half (the absent half is memset to zero in
                # SBUF), cutting the zero-padding out of the DMA stream
                pairs = [e for e in cl if nzb_top[e[0], e[1]] and nzb_bot[e[0], e[1]]]
                tops = [e for e in cl if nzb_top[e[0], e[1]] and not nzb_bot[e[0], e[1]]]
                bots = [e for e in cl if not nzb_top[e[0], e[1]]]
                order = pairs + tops + bots
                chunks[r2] = ("compact", strip_cols[r2], len(pairs),
                              single_cols[r2], len(tops), len(bots))
                strip_cols[r2] += len(pairs) * BS
                single_cols[r2] += (len(tops) + len(bots)) * BS
            for k, (I, j) in enumerate(order):
                c = j % 4
                queues.setdefault((r2, c), []).append((r2, c, k * BS, I))
        for r2 in range(2):
            for c in range(4):
                if (r2, c) not in queues:
                    queues[(r2, c)] = [(r2, c, None, 0)]
        # Round-robin across the 8 sub-array positions for concurrency.
        sched = []
        rounds = []
        qlists = [queues[k] for k in sorted(queues.keys())]
        idx = [0] * len(qlists)
        remaining = sum(len(q) for q in qlists)
        while remaining:
            rnd = []
            for qi, q in enumerate(qlists):
                if idx[qi] < len(q):
                    r2, c, woff, I = q[idx[qi]]
                    start = idx[qi] == 0
                    stop = idx[qi] == len(q) - 1
                    e = (r2, c, woff, I, start, stop)
                    sched.append(e)
                    rnd.append(e)
                    idx[qi] += 1
                    remaining -= 1
            rounds.append(rnd)
        plan.append({"chunks": chunks, "sched": sched, "rounds": rounds})
    return plan, strip_cols, single_cols


def _strip_reused_ldweights(nc, pairs):
    """Delete the scheduler-generated InstLdweights immediately preceding
    each 'reuse' matmul.

    pairs: list of (keeper_mm_name, reuse_mm_name).  The reuse matmul sits
    at the same PE tile position as the keeper with an identical stationary
    AP and no intervening LDWEIGHTS at that position, so the hardware's
    per-position stationary weights are still valid (verified on hw).
    Any waits/updates on the deleted LDW are transferred to the matmul.
    """
    import concourse.mybir as mybir

    reuse_of = {r: k for (k, r) in pairs}
    removed = set()
    for fn in nc.m.functions:
        for blk in fn.blocks:
            ilist = list(blk.instructions)
            name_to_idx = {inst.name: i for i, inst in enumerate(ilist)}

            def ldw_before(i):
                j = i - 1
                while j >= 0 and not isinstance(
                    ilist[j], (mybir.InstLdweights, mybir.InstMatmult)
                ):
                    j -= 1
                assert j >= 0 and isinstance(ilist[j], mybir.InstLdweights), (
                    f"no LDWEIGHTS before {ilist[i].name}"
                )
                return j

            to_del = []
            for reuse_nm, keep_nm in reuse_of.items():
                i = name_to_idx.get(reuse_nm)
                if i is None:
                    continue
                mm = ilist[i]
                j = ldw_before(i)
                ldw = ilist[j]
                k = name_to_idx.get(keep_nm)
                assert k is not None and k < j, (reuse_nm, keep_nm)
                jk = ldw_before(k)
                ldw_keep = ilist[jk]
                # safety: identical position and stationary source
                assert ldw.tile_position == ldw_keep.tile_position, (
                    ldw.tile_position, ldw_keep.tile_position)
                assert ldw.tile_size == ldw_keep.tile_size
                a, b = ldw.ins[0], ldw_keep.ins[0]
                assert a.offset == b.offset and str(a) == str(b), (
                    reuse_nm, str(a), str(b))
                si = ldw.sync_info
                if si is not None and (len(si.on_wait) or len(si.on_update)):
                    msi = mm.sync_info
                    ow = list(si.on_wait) + (list(msi.on_wait) if msi else [])
                    ou = list(si.on_update) + (list(msi.on_update) if msi else [])
                    mm.sync_info = mybir.SyncInfo(on_wait=ow, on_update=ou)
                to_del.append(j)
                removed.add(ldw.name)
            for j in sorted(to_del, reverse=True):
                del blk.instructions[j]
            if removed:
                for inst in blk.instructions:
                    try:
                        edges = inst.dependency_edges()
                    except Exception:
                        continue
                    for edge in edges:
                        n = edge[0]
                        if n in removed:
                            inst.try_remove_dependency(n)
    return len(removed)


def _chunk_width(ch):
    """SBUF columns a (J, r2) strip occupies."""
    if ch[0] == "legacy":
        return ch[2] * BS
    return (ch[2] + ch[4] + ch[5]) * BS


def _build_program(plan, strip_cols, single_cols):
    import concourse.bacc as bacc
    import concourse.tile as tile
    import concourse.mybir as mybir

    nc = bacc.Bacc(debug=False)
    bf16, f32 = mybir.dt.bfloat16, mybir.dt.float32

    xt_d = nc.declare_dram_parameter(
        "xt", [N_MSL * (N_T // 2), 128, 2 * MSL], bf16, isOutput=False
    )
    w_d = {}
    ws_d = {}
    for r2 in range(2):
        if strip_cols[r2] > 0:
            w_d[r2] = nc.declare_dram_parameter(
                f"w{r2}", [2 * BS, strip_cols[r2]], bf16, isOutput=False
            )
        if single_cols[r2] > 0:
            ws_d[r2] = nc.declare_dram_parameter(
                f"ws{r2}", [BS, single_cols[r2]], bf16, isOutput=False
            )
    out_d = nc.declare_dram_parameter("out", [OUT_F, M_CORE], bf16, isOutput=True)

    # Largest per-(J, strip) weight chunk, in columns (>= BS for the tile alloc).
    lmax = BS
    for p in plan:
        for r2 in range(2):
            lmax = max(lmax, _chunk_width(p["chunks"][r2]))

    reuse_pairs = []  # (keeper_mm_name, reuse_mm_name) for LDW dedup

    with tile.TileContext(nc) as tc:
        with (
            tc.tile_pool(name="xp", bufs=1) as xp,
            tc.tile_pool(name="zp", bufs=1) as zp,
            tc.tile_pool(name="wr", bufs=1) as wr,     # resident ramp weights
            tc.tile_pool(name="wp", bufs=6) as wp,     # streaming phase-2 weights
            tc.tile_pool(name="ep", bufs=16) as ep,
            tc.tile_pool(name="pp", bufs=4, space="PSUM") as pp,
        ):
            wts = {}

            def load_w(J, eng0=None, eng1=None):
                if J < N_PRE:
                    width = max(BS, max(_chunk_width(plan[J]["chunks"][r2])
                                        for r2 in range(2)))
                    wt = wr.tile([128, width], bf16, tag=f"w{J}")
                else:
                    wt = wp.tile([128, lmax], bf16, tag="wt")
                engs = (eng0 or nc.sync, eng1 or eng0 or nc.scalar)
                # zero the absent halves of half-present cells before the DMAs
                for r2 in range(2):
                    ch = plan[J]["chunks"][r2]
                    if ch[0] == "compact" and ch[4] + ch[5]:
                        nc.vector.memset(
                            wt[64 * r2 : 64 * r2 + 64,
                               ch[2] * BS : (ch[2] + ch[4] + ch[5]) * BS],
                            0.0,
                        )
                for r2 in range(2):
                    ch = plan[J]["chunks"][r2]
                    if ch[0] == "legacy":
                        _, base, ncell = ch
                        if ncell:
                            engs[r2].dma_start(
                                wt[64 * r2 : 64 * r2 + 64, : ncell * BS],
                                w_d[r2][:, base : base + ncell * BS],
                            )
                        continue
                    _, pbase, npair, sbase, ntop, nbot = ch
                    if npair:
                        engs[r2].dma_start(
                            wt[64 * r2 : 64 * r2 + 64, : npair * BS],
                            w_d[r2][:, pbase : pbase + npair * BS],
                        )
                    if ntop:
                        engs[r2].dma_start(
                            wt[64 * r2 : 64 * r2 + 32,
                               npair * BS : (npair + ntop) * BS],
                            ws_d[r2][:, sbase : sbase + ntop * BS],
                        )
                    if nbot:
                        engs[r2].dma_start(
                            wt[64 * r2 + 32 : 64 * r2 + 64,
                               (npair + ntop) * BS : (npair + ntop + nbot) * BS],
                            ws_d[r2][:, sbase + ntop * BS
                                     : sbase + (ntop + nbot) * BS],
                        )
                wts[J] = wt
                return wt

            Xc = {}

            def load_x_chunk(t, m, eng):
                # one DMA per chunk PAIR (2t, 2t+1): per-partition source runs
                # are 2 KB contiguous (host stores pairs adjacently), halving
                # descriptor count vs per-chunk loads
                tp = t // 2
                if (tp, m) in Xc:
                    return
                xchunk = xp.tile([128, 2 * MSL], bf16, tag=f"x{tp}_{m}")
                Xc[(tp, m)] = xchunk
                eng.dma_start(xchunk[:], xt_d[m * (N_T // 2) + tp])

            # DMA emission order.  Per-queue FIFO plan (queues measured at
            # ~150-240 GB/s each; HBM ~330 GB/s aggregate):
            #   sync:   GEN0 heads | GEN0 tails(J0,1) | w4-7  | xm1 tail | w12,14,..
            #   scalar: xm0 evens  | GEN0 tail(J2)    | w8-11 | xm1 share | w13,15,..
            #   gpsimd: xm0 odds   | GEN0 tail(J3)    | xm1 bulk | outputs
            # GEN0 strips are split into a head (the cells the chunk-major
            # ramp consumes first) + remainder so the first matmuls' weights
            # land ASAP.
            heads = {}

            def load_w_head(J, ncut):
                width = max(BS, max(_chunk_width(plan[J]["chunks"][r2])
                                    for r2 in range(2)))
                wt = wr.tile([128, width], bf16, tag=f"w{J}")
                for r2 in range(2):
                    _, base, ncell = plan[J]["chunks"][r2]
                    n0 = min(ncut, ncell)
                    heads[(J, r2)] = n0
                    if n0:
                        nc.sync.dma_start(
                            wt[64 * r2 : 64 * r2 + 64, : n0 * BS],
                            w_d[r2][:, base : base + n0 * BS],
                        )
                wts[J] = wt
                return wt

            def load_w_tail(J, eng):
                wt = wts[J]
                for r2 in range(2):
                    _, base, ncell = plan[J]["chunks"][r2]
                    n0 = heads[(J, r2)]
                    if ncell > n0:
                        eng.dma_start(
                            wt[64 * r2 : 64 * r2 + 64, n0 * BS : ncell * BS],
                            w_d[r2][:, base + n0 * BS : base + ncell * BS],
                        )

            for J in range(4):
                load_w_head(J, 20)
            for tp in range(0, 4):
                load_x_chunk(2 * tp, 0, (nc.scalar, nc.gpsimd)[tp % 2])
            load_w_tail(0, nc.sync)
            load_w_tail(1, nc.sync)
            for tp in range(4, 6):
                load_x_chunk(2 * tp, 0, (nc.scalar, nc.gpsimd)[tp % 2])
            load_w_tail(2, nc.scalar)
            load_w_tail(3, nc.gpsimd)
            for tp in range(6, N_T // 2):
                load_x_chunk(2 * tp, 0, (nc.scalar, nc.gpsimd)[tp % 2])
            zw = zp.tile([128, BS], bf16)
            nc.vector.memset(zw[:], 0.0)
            # ramp weights: 2-queue round-robin, both strips of a tile on the
            # same queue so tiles complete in J order
            for J in range(4, N_PRE):
                eng = (nc.sync, nc.scalar)[J % 2]
                load_w(J, eng, eng)
            # x m1: bulk on gpsimd (free after xm0), tail shared by all three
            for tp in range(0, 12):
                load_x_chunk(2 * tp, 1, nc.gpsimd)
            for tp in range(12, N_T // 2):
                load_x_chunk(2 * tp, 1, (nc.sync, nc.scalar, nc.gpsimd)[tp % 3])
            # first two phase-2 weight tiles early, so the fused phase doesn't
            # wait on them right after the ramp
            load_w(N_PRE, nc.sync, nc.sync)
            load_w(N_PRE + 1, nc.scalar, nc.scalar)
            # remaining phase-2 weights are emitted just-in-time inside the
            # loop below (wp pool rotation paces them ~6 supertiles ahead).

            def emit_mm(P, wt, r2, c, woff, I, m, start, stop):
                lhsT = (
                    zw[64 * r2 : 64 * r2 + 64, :]
                    if woff is None
                    else wt[64 * r2 : 64 * r2 + 64, woff : woff + BS]
                )
                tlo = (I // 2) % 2
                return nc.tensor.matmul(
                    P[32 * c : 32 * c + 32, r2, :],
                    lhsT,
                    Xc[(I // 4, m)][64 * r2 : 64 * r2 + 64,
                                    tlo * MSL : (tlo + 1) * MSL],
                    start=start,
                    stop=stop,
                    tile_position=(64 * r2, 32 * c),
                )

            def emit_evac(P, J, m, eng=None):
                # ob is bf16: halves output DMA traffic; the final-value
                # rounding adds ~1e-3 relative error (budget is 2e-2).
                ob = ep.tile([128, MSL], bf16, tag="ob")
                with nc.allow_low_precision(
                    reason="final-value bf16 rounding; rel err budget 2e-2"
                ):
                    nc.vector.reduce_sum(
                        ob[:], P[:].transpose([0, 2, 1]), axis=mybir.AxisListType.X
                    )
                if eng is None:
                    eng = nc.gpsimd
                eng.dma_start(
                    out_d[128 * J : 128 * (J + 1), m * MSL : (m + 1) * MSL],
                    ob[:],
                )

            # --- phase 1a: ramp generation, J=0..3 m0, chunk-major merged ---
            # All four tiles' blocks for x chunk t run before any of chunk
            # t+1, using all 4 PSUM slots, so compute tracks x-chunk arrival
            # instead of head-of-line blocking on one tile's late chunks.
            GEN0 = list(range(min(4, N_PRE)))
            merged = []
            for J in GEN0:
                for k, (r2, c, woff, I, _s0, _s1) in enumerate(plan[J]["sched"]):
                    t = -1 if woff is None else I // 2
                    merged.append((t, k, J, r2, c, woff, I))
            merged.sort(key=lambda e: (e[0], e[1], e[2]))
            first_of = {}
            last_of = {}
            for idx, e in enumerate(merged):
                key = (e[2], e[3], e[4])
                first_of.setdefault(key, idx)
                last_of[key] = idx

            P_gen = {J: pp.tile([128, 2, MSL], f32, tag="P", name=f"Pg{J}")
                     for J in GEN0}
            for idx, (t, k, J, r2, c, woff, I) in enumerate(merged):
                key = (J, r2, c)
                emit_mm(
                    P_gen[J], wts[J], r2, c, woff, I, 0,
                    first_of[key] == idx, last_of[key] == idx,
                )
            for J in GEN0:
                emit_evac(P_gen[J], J, 0)

            # --- phase 1b: ramp J=4..N_PRE-1, m0 only ---
            for J in range(len(GEN0), N_PRE):
                P = pp.tile([128, 2, MSL], f32, tag="P")
                for r2, c, woff, I, start, stop in plan[J]["sched"]:
                    emit_mm(P, wts[J], r2, c, woff, I, 0, start, stop)
                emit_evac(P, J, 0)

            # --- phase 2: J=N_PRE..N_J-1, fused m0+m1 with stationary reuse.
            # Per round-robin round: all positions' m0 matmuls, then the same
            # entries' m1 matmuls (same stationary weights; the second LDW is
            # deleted after scheduling).  Same-position instructions stay ~8
            # apart so the in-order PE queue never waits on a busy sub-array.
            for J in range(N_PRE, N_J):
                if J in wts:
                    wt = wts[J]
                else:
                    eng = (nc.sync, nc.scalar)[J % 2]
                    wt = load_w(J, eng, eng)
                P0 = pp.tile([128, 2, MSL], f32, tag="P")
                P1 = pp.tile([128, 2, MSL], f32, tag="P")
                rounds = plan[J]["rounds"]
                for ri, rnd in enumerate(rounds):
                    mm0 = [emit_mm(P0, wt, r2, c, woff, I, 0, start, stop)
                           for (r2, c, woff, I, start, stop) in rnd]
                    if ri == len(rounds) - 1:
                        emit_evac(P0, J, 0)
                    mm1 = [emit_mm(P1, wt, r2, c, woff, I, 1, start, stop)
                           for (r2, c, woff, I, start, stop) in rnd]
                    for a, b in zip(mm0, mm1):
                        reuse_pairs.append((a.ins.name, b.ins.name))
                emit_evac(P1, J, 1)

            # --- phase 3: ramp tiles' m1, weights still resident in SBUF ---
            for J in range(N_PRE):
                P = pp.tile([128, 2, MSL], f32, tag="P")
                for r2, c, woff, I, start, stop in plan[J]["sched"]:
                    emit_mm(P, wts[J], r2, c, woff, I, 1, start, stop)
                emit_evac(P, J, 1)

    n = _strip_reused_ldweights(nc, reuse_pairs)
    assert n == len(reuse_pairs), (n, len(reuse_pairs))
    nc.compile()
    return nc


_CACHE = {}


def kernel(x, W, bias, mask):
    assert x.shape == (B, S, IN_F) and W.shape == (IN_F, OUT_F)
    _ensure_ntff_hook()
    from concourse.bass_utils import run_bass_kernel_spmd

    # --- host-side input prep -------------------------------------------
    mask_nz = mask != 0
    nzb = np.asarray(mask_nz.reshape(GI, BS, GJ, BS).any(axis=(1, 3)))

    key = nzb.tobytes()
    if key not in _CACHE:
        perm = _pair_permutation(nzb)
        plan, strip_cols, single_cols = _plan(nzb[perm])
        nc = _build_program(plan, strip_cols, single_cols)
        _CACHE[key] = (perm, plan, strip_cols, single_cols, nc)
    perm, plan, strip_cols, single_cols, nc = _CACHE[key]
    nzb_p = nzb[perm]

    # Masked weights, gathered per row strip in storage order (J-major),
    # mirroring _plan's per-(J, r2) cell order exactly.
    Wm = np.where(mask_nz, W, np.float32(0)).astype(np.float32)
    W4 = Wm.reshape(GI, BS, GJ, BS)  # block (i, j) = W4[i, :, j, :]
    nzb_top, nzb_bot = nzb_p[0::2], nzb_p[1::2]
    nzb2 = nzb_top | nzb_bot
    strips = {}
    sstrips = {}
    for r2 in range(2):
        pair_II, pair_JJ = [], []      # 64-row panels (legacy + pairs)
        sing_II, sing_JJ = [], []      # 32-row panels (block-row index)
        for J in range(N_J):
            cl = [(I, j)
                  for I in range(GP) if I % 2 == r2
                  for j in range(J * JCOLS, (J + 1) * JCOLS) if nzb2[I, j]]
            if J < 4:
                for I, j in cl:
                    pair_II.append(I)
                    pair_JJ.append(j)
            else:
                for I, j in cl:
                    if nzb_top[I, j] and nzb_bot[I, j]:
                        pair_II.append(I)
                        pair_JJ.append(j)
                for I, j in cl:
                    if nzb_top[I, j] and not nzb_bot[I, j]:
                        sing_II.append(perm[2 * I])
                        sing_JJ.append(j)
                for I, j in cl:
                    if not nzb_top[I, j]:
                        sing_II.append(perm[2 * I + 1])
                        sing_JJ.append(j)
        if pair_II:
            II = np.asarray(pair_II)
            JJ = np.asarray(pair_JJ)
            top = W4[perm[2 * II], :, JJ, :]       # [n, 32, 32]
            bot = W4[perm[2 * II + 1], :, JJ, :]   # [n, 32, 32]
            panel = np.concatenate([top, bot], axis=1)  # [n, 64, 32]
            strips[r2] = np.ascontiguousarray(
                panel.transpose(1, 0, 2).reshape(2 * BS, -1)
            ).astype(BF16)
        if sing_II:
            pan = W4[np.asarray(sing_II), :, np.asarray(sing_JJ), :]  # [n,32,32]
            sstrips[r2] = np.ascontiguousarray(
                pan.transpose(1, 0, 2).reshape(BS, -1)
            ).astype(BF16)
        assert strips.get(r2, np.zeros((1, 0))).shape[1] == strip_cols[r2]
        assert sstrips.get(r2, np.zeros((1, 0))).shape[1] == single_cols[r2]

    xf = np.ascontiguousarray(x).reshape(B * S, IN_F)
    in_maps = []
    for c in range(N_CORES):
        xt = np.ascontiguousarray(
            xf[c * M_CORE : (c + 1) * M_CORE].T
        ).astype(BF16)
        xt = xt.reshape(GI, BS, M_CORE)[perm].reshape(IN_F, M_CORE)
        # [m][t//2][p][(t%2)*MSL + q]: chunk pairs adjacent per partition so
        # one DMA covers two chunks with 2KB-contiguous per-partition runs
        xtc = (
            xt.reshape(N_T // 2, 2, 128, N_MSL, MSL)
            .transpose(3, 0, 2, 1, 4)
            .reshape(N_MSL * (N_T // 2), 128, 2 * MSL)
        )
        m = {"xt": np.ascontiguousarray(xtc)}
        for r2, arr in strips.items():
            m[f"w{r2}"] = arr
        for r2, arr in sstrips.items():
            m[f"ws{r2}"] = arr
        in_maps.append(m)

    # --- run -------------------------------------------------------------
    res = run_bass_kernel_spmd(nc, in_maps, list(range(N_CORES)), trace=True)

    # --- host-side output assembly --------------------------------------
    y = np.empty((B * S, OUT_F), dtype=np.float32)
    for c in range(N_CORES):
        y[c * M_CORE : (c + 1) * M_CORE] = res.results[c]["out"].astype(np.float32).T
    y = y.reshape(B, S, OUT_F)
    if np.any(bias):
        # bias is all-zero in this problem's setup; handled host-side for
        # generality.
        y = y + bias.astype(np.float32)
    kernel.last_exec_time_ns = res.exec_time_ns
    return y
